# revision 1
# baseline (speedup 1.0000x reference)
# Self-contained Trainium2 Bass kernel for nn_AttentionBlock (AdaLN + QK-norm attention).
#
# Sharding: 8 cores = 4 batches (data parallel) x 2 head-groups of 8 heads
# (tensor parallel).  Each core computes:
#   scale_shift = emb_b @ W_emb + b_emb                  (full, per batch)
#   xn^T        = (rmsnorm(x_b) * (1+scale) + shift)^T   (dim on partitions)
#   q^T,k^T     = (xn @ Wq_g)^T, (xn @ Wk_g)^T           [d*8, n]
#   v           = xn @ Wv_g                              [n, d*8] (token layout)
#   per head: E^T = exp(qk^T scaled), o^T = v^T E / colsum(E)
#   out_part    = o @ (W_out_rows + I_rows)              [n, dim]  (residual folded)
# Host sums the two head-group partials per batch.
import numpy as np

B, N, DIM = 4, 2048, 2048
H_TOT, D = 16, 128
HG = 2                # head groups (tensor-parallel)
H = H_TOT // HG       # heads per core = 8
QK = H * D            # 1024 q (or k) columns per core
C3 = 3 * QK           # 3072 qkv columns per core
KC = DIM // 128       # 16 dim chunks
TC = N // 128         # 16 token chunks
EPS = 1e-6
NCORES = 8

_COMPILED = None


def _build(stop_after=None):
    import concourse.bass as bass
    import concourse.bacc as bacc
    import concourse.tile as tile
    from concourse import mybir
    from concourse.masks import make_identity

    f32 = mybir.dt.float32
    bf16 = mybir.dt.bfloat16
    AF = mybir.ActivationFunctionType
    OP = mybir.AluOpType

    nc = bacc.Bacc(
        "TRN2", target_bir_lowering=False, debug=False, num_devices=NCORES
    )

    # ---- DRAM I/O -------------------------------------------------------
    x_b = nc.dram_tensor("x_b", [N, DIM], f32, kind="ExternalInput").ap()
    mcol_in = nc.dram_tensor("mcol_in", [128, KC], f32, kind="ExternalInput").ap()
    scol_in = nc.dram_tensor("scol_in", [128, KC], f32, kind="ExternalInput").ap()
    gqk_in = nc.dram_tensor("gqk_in", [D], f32, kind="ExternalInput").ap()
    W_qkv_s = nc.dram_tensor("W_qkv_s", [DIM, C3], f32, kind="ExternalInput").ap()
    W_out_s = nc.dram_tensor("W_out_s", [QK, DIM], f32, kind="ExternalInput").ap()
    out_p = nc.dram_tensor("out_p", [N, DIM], f32, kind="ExternalOutput").ap()
    o_d = nc.dram_tensor("o_d", [128, H * N], bf16).ap()  # o^T bounce [p, h*n]

    with tile.TileContext(nc) as tc:
        _emit(nc, tc, bass, mybir, tile, make_identity, f32, bf16, AF, OP,
              x_b, mcol_in, scol_in, gqk_in, W_qkv_s, W_out_s,
              out_p, o_d, stop_after)
    nc.compile()
    return nc


def _emit(nc, tc, bass, mybir, tile, make_identity, f32, bf16, AF, OP,
          x_b, mcol_in, scol_in, gqk_in, W_qkv_s, W_out_s,
          out_p, o_d, stop_after=None):
    from contextlib import ExitStack

    ts = bass.ts

    with ExitStack() as ctx:
        consts = ctx.enter_context(tc.tile_pool(name="consts", bufs=1))

        ident = consts.tile([128, 128], bf16)
        make_identity(nc, ident)
        ones_col = consts.tile([128, 1], bf16)
        nc.vector.memset(ones_col, 1.0)
        ones_row = consts.tile([1, 128], bf16)
        nc.vector.memset(ones_row, 1.0)

        mcol = consts.tile([128, KC], f32)          # g_norm*(1+scale) as columns
        nc.sync.dma_start(out=mcol, in_=mcol_in)
        scol = consts.tile([128, KC], f32)          # shift as columns
        nc.sync.dma_start(out=scol, in_=scol_in)
        gqk_f = consts.tile([1, D], f32)
        nc.sync.dma_start(out=gqk_f, in_=gqk_in.rearrange("(a n) -> a n", a=1))
        gqk = consts.tile([1, D], bf16)             # g_q * g_k (fold both on q side)
        nc.vector.tensor_copy(gqk, gqk_f)
        eps128 = consts.tile([128, 1], f32)
        nc.vector.memset(eps128, EPS)
        epsq1 = consts.tile([1, 1], f32)
        nc.vector.memset(epsq1, D * EPS)
        epsk1 = consts.tile([1, 1], f32)
        nc.vector.memset(epsk1, EPS)
        if stop_after == 'ph0':
            return
        # ---- persistent q^T / k^T / v ---------------------------------
        with tc.tile_pool(name="qkv", bufs=1) as qkv:
            qT = qkv.tile([128, H, N], bf16)
            kT = qkv.tile([128, H, N], bf16)
            vS = qkv.tile([128, TC, QK], bf16)

            # ======== Phase 1 + 2a interleaved =========================
            with tc.tile_pool(name="xnp", bufs=1) as xnp:
                xnT = xnp.tile([128, KC, N], bf16)
                wr = W_qkv_s.rearrange("(c p) n -> p c n", p=128)

                def emit_ph1_group(ph1, ph1s, ph1sq, ph1ps, tg, split=False):
                    if split:
                        # two 2-tile subgroups: halves the first-eviction gate
                        for half in range(2):
                            xts, dgs = [], []
                            for tt in range(2):
                                t = tg * 4 + half * 2 + tt
                                xt = ph1.tile([128, DIM], bf16, tag="xt",
                                              name=f"xts{t}")
                                nc.gpsimd.dma_start(out=xt, in_=x_b[ts(t, 128), :])
                                sq = ph1sq.tile([128, DIM], bf16, tag="sq",
                                                name=f"sqs{t}")
                                ssq = ph1s.tile([128, 1], f32, tag="ssq",
                                                name=f"ssqs{t}")
                                nc.scalar.activation(sq, xt, AF.Square,
                                                     accum_out=ssq)
                                rin = ph1s.tile([128, 1], f32, tag="rin",
                                                name=f"ris{t}")
                                nc.scalar.activation(rin, ssq, AF.Sqrt,
                                                     scale=1.0 / DIM,
                                                     bias=eps128)
                                rr = ph1s.tile([128, 1], f32, tag="rr",
                                                name=f"rrs{t}")
                                nc.vector.reciprocal(rr, rin)
                                diag = ph1s.tile([128, 128], bf16, tag="dg",
                                                 name=f"dgs{t}")
                                nc.vector.tensor_scalar_mul(diag, ident, rr)
                                xts.append(xt)
                                dgs.append(diag)
                            for c in range(KC):
                                pst = ph1ps.tile([128, 256], f32, tag="pt",
                                                 name=f"pts{half}_{c}")
                                for tt in range(2):
                                    nc.tensor.matmul(pst[:, ts(tt, 128)],
                                                     xts[tt][:, ts(c, 128)],
                                                     dgs[tt],
                                                     start=True, stop=True)
                                nc.vector.tensor_scalar(
                                    out=xnT[:, c,
                                            tg * 512 + half * 256 :
                                            tg * 512 + (half + 1) * 256],
                                    in0=pst,
                                    scalar1=mcol[:, c : c + 1],
                                    scalar2=scol[:, c : c + 1],
                                    op0=OP.mult, op1=OP.add,
                                )
                        return
                    xts, dgs = [], []
                    for tt in range(4):
                        t = tg * 4 + tt
                        xt = ph1.tile([128, DIM], bf16, tag="xt", name=f"xt{t}")
                        nc.gpsimd.dma_start(out=xt, in_=x_b[ts(t, 128), :])
                        sq = ph1sq.tile([128, DIM], bf16, tag="sq", name=f"sq{t}")
                        ssq = ph1s.tile([128, 1], f32, tag="ssq", name=f"ssq{t}")
                        nc.scalar.activation(sq, xt, AF.Square, accum_out=ssq)
                        rin = ph1s.tile([128, 1], f32, tag="rin", name=f"ri{t}")
                        nc.scalar.activation(rin, ssq, AF.Sqrt,
                                             scale=1.0 / DIM, bias=eps128)
                        rr = ph1s.tile([128, 1], f32, tag="rr", name=f"rr{t}")
                        nc.vector.reciprocal(rr, rin)
                        diag = ph1s.tile([128, 128], bf16, tag="dg", name=f"dg{t}")
                        nc.vector.tensor_scalar_mul(diag, ident, rr)
                        xts.append(xt)
                        dgs.append(diag)
                    for c in range(KC):
                        pst = ph1ps.tile([128, 512], f32, tag="pt", name=f"pt{tg}_{c}")
                        for tt in range(4):
                            nc.tensor.matmul(pst[:, ts(tt, 128)],
                                             xts[tt][:, ts(c, 128)],
                                             dgs[tt],
                                             start=True, stop=True)
                        nc.vector.tensor_scalar(
                            out=xnT[:, c, ts(tg, 512)], in0=pst,
                            scalar1=mcol[:, c : c + 1],
                            scalar2=scol[:, c : c + 1],
                            op0=OP.mult, op1=OP.add,
                        )

                def emit_qk_mchunk(ph2w, ph2ps, m, nts, half, wm=None,
                                    fine_first=False):
                    if wm is None:
                        wm = ph2w.tile([128, KC, 128], bf16, tag="wqk",
                                       name=f"wm{half}_{m}")
                        nc.gpsimd.dma_start(out=wm, in_=wr[:, :, ts(m, 128)])
                    dst = qT if m < H else kT
                    hh = m if m < H else m - H
                    if fine_first:
                        # start on the first 256 tokens as soon as they land
                        for piece in range(2):
                            ps = ph2ps.tile([128, 256], f32, tag="qkps",
                                            name=f"qkf{m}_{piece}")
                            for k in range(KC):
                                nc.tensor.matmul(
                                    ps, wm[:, k, :],
                                    xnT[:, k, ts(piece, 256)],
                                    start=(k == 0), stop=(k == KC - 1),
                                )
                            nc.vector.tensor_copy(
                                dst[:, hh, ts(piece, 256)], ps)
                        nts = tuple(n for n in nts if n != 0)
                    for nt in nts:
                        ps = ph2ps.tile([128, 512], f32, tag="qkps", name=f"qk{half}_{m}_{nt}")
                        for k in range(KC):
                            nc.tensor.matmul(
                                ps, wm[:, k, :], xnT[:, k, ts(nt, 512)],
                                start=(k == 0), stop=(k == KC - 1),
                            )
                        nc.vector.tensor_copy(dst[:, hh, ts(nt, 512)], ps)

                def emit_ph25_head(p25, p25r, p25ps, p25bc, h):
                    for which in range(2):   # 0: q, 1: k
                        src = qT if which == 0 else kT
                        sq = p25.tile([128, N], bf16, tag="sq25",
                                      name=f"sq25_{h}_{which}")
                        nc.scalar.activation(sq, src[:, h, :], AF.Square)
                        rv = p25r.tile([1, N], bf16, tag="rv",
                                       name=f"rv{h}_{which}")
                        for nt in range(4):
                            pss = p25ps.tile([1, 512], f32, tag="sm")
                            nc.tensor.matmul(pss, ones_col, sq[:, ts(nt, 512)],
                                             start=True, stop=True)
                            if which == 0:
                                nc.scalar.activation(
                                    rv[:, ts(nt, 512)], pss, AF.Sqrt,
                                    bias=epsq1)
                            else:
                                nc.scalar.activation(
                                    rv[:, ts(nt, 512)], pss, AF.Sqrt,
                                    scale=1.0 / D, bias=epsk1)
                        with nc.allow_low_precision(reason="bf16 rms scale"):
                            nc.vector.reciprocal(rv, rv)
                        lhs = gqk if which == 0 else ones_row
                        for nt in range(4):
                            pb = p25bc.tile([128, 512], f32, tag="bc")
                            nc.tensor.matmul(pb, lhs, rv[:, ts(nt, 512)],
                                             start=True, stop=True)
                            nc.vector.tensor_mul(
                                src[:, h, ts(nt, 512)],
                                src[:, h, ts(nt, 512)], pb)

                # q0,k0,q1,k1,... order so each head pair completes together
                m_order = [x for h in range(H) for x in (h, H + h)]

                with tc.tile_pool(name="ph1", bufs=4) as ph1, \
                     tc.tile_pool(name="ph1s", bufs=4) as ph1s, \
                     tc.tile_pool(name="ph1sq", bufs=1) as ph1sq, \
                     tc.tile_pool(name="ph2w", bufs=3) as ph2w, \
                     tc.tile_pool(name="p25", bufs=1) as p25, \
                     tc.tile_pool(name="p25r", bufs=1) as p25r, \
                     tc.tile_pool(name="ph1ps", bufs=2, space="PSUM") as ph1ps, \
                     tc.tile_pool(name="ph2ps", bufs=2, space="PSUM") as ph2ps, \
                     tc.tile_pool(name="p25ps", bufs=2, space="PSUM") as p25ps, \
                     tc.tile_pool(name="p25bc", bufs=2, space="PSUM") as p25bc:
                    # prefetch first two W chunks ahead of the x loads so
                    # the first qk matmuls aren't gated on the SWDGE queue
                    emit_ph1_group(ph1, ph1s, ph1sq, ph1ps, 0, split=True)
                    emit_ph1_group(ph1, ph1s, ph1sq, ph1ps, 1, split=True)
                    for i, m in enumerate(m_order):
                        emit_qk_mchunk(ph2w, ph2ps, m, (0, 1), 0,
                                       fine_first=(i == 0))
                        if i == 3:
                            emit_ph1_group(ph1, ph1s, ph1sq, ph1ps, 2,
                                           split=True)
                        if i == 9:
                            emit_ph1_group(ph1, ph1s, ph1sq, ph1ps, 3,
                                           split=True)
                    for i, m in enumerate(m_order):
                        emit_qk_mchunk(ph2w, ph2ps, m, (2, 3), 1)
                        if i % 2 == 1:
                            emit_ph25_head(p25, p25r, p25ps, p25bc, m - H)

                if stop_after == 'ph2a':
                    return
                # ======== Phase 2b: v (token-major) ====================
                if stop_after == 'ph2a':
                    return
                with tc.tile_pool(name="ph2v", bufs=2) as ph2v, \
                     tc.tile_pool(name="ph2vps", bufs=4, space="PSUM") as ph2vps:
                    for nv in range(4):      # 4 slabs of 256 v-columns
                        wv = ph2v.tile([128, KC, 256], bf16, tag="wv")
                        nc.gpsimd.dma_start(
                            out=wv,
                            in_=wr[:, :, 2 * QK + nv * 256 : 2 * QK + (nv + 1) * 256],
                        )
                        for t in range(TC):
                            ps = ph2vps.tile([128, 256], f32)
                            for k in range(KC):
                                nc.tensor.matmul(
                                    ps, xnT[:, k, ts(t, 128)], wv[:, k, :],
                                    start=(k == 0), stop=(k == KC - 1),
                                )
                            nc.scalar.copy(vS[:, t, ts(nv, 256)], ps)

            # ---- W_out' prefetch pool; chunks loaded during attention --
            ph4w = ctx.enter_context(
                tc.tile_pool(name="ph4w", bufs=1, side="right"))
            Wp = ph4w.tile([128, H, DIM], bf16)
            wor = W_out_s.rearrange("(c p) n -> p c n", p=128)

            # ======== Phase 3: attention ===============================
            with tc.tile_pool(name="Ep", bufs=1) as Ep, \
                 tc.tile_pool(name="P2p", bufs=1) as P2p, \
                 tc.tile_pool(name="at", bufs=2) as at, \
                 tc.tile_pool(name="atr", bufs=2) as atr, \
                 tc.tile_pool(name="lps", bufs=2, space="PSUM") as lps, \
                 tc.tile_pool(name="ops", bufs=2, space="PSUM") as ops, \
                 tc.tile_pool(name="dps", bufs=2, space="PSUM") as dps:
                E = Ep.tile([128, KC, 1024], bf16)
                nc.gpsimd.dma_start(out=Wp, in_=wor)
                for h in range(H):
                    for qh in range(2):      # q halves of 1024
                        q0 = qh * 1024
                        P2 = P2p.tile([128, KC // 2, 1024], bf16, tag="p2",
                                      name=f"P2_{h}_{qh}")
                        for kc in range(KC):
                            pl = lps.tile([128, 1024], f32, tag="lg")
                            for j in range(2):
                                nc.tensor.matmul(
                                    pl[:, ts(j, 512)],
                                    kT[:, h, ts(kc, 128)],
                                    qT[:, h, q0 + j * 512 : q0 + (j + 1) * 512],
                                    start=True, stop=True,
                                )
                            nc.scalar.activation(E[:, kc, :], pl, AF.Exp)
                            if kc % 2 == 1:
                                # binary-tree partial sums for the softmax
                                # denominator (DVE, bf16 2x): leaves E pairs,
                                # upper levels folded in place into P2 so the
                                # PE pass shrinks to a single matmul.
                                j = kc // 2
                                nc.vector.tensor_add(
                                    P2[:, j, :],
                                    E[:, kc - 1, :], E[:, kc, :])
                                if j % 2 == 1:
                                    nc.vector.tensor_add(
                                        P2[:, j - 1, :],
                                        P2[:, j - 1, :], P2[:, j, :])
                                if j % 4 == 3:
                                    nc.vector.tensor_add(
                                        P2[:, j - 3, :],
                                        P2[:, j - 3, :], P2[:, j - 1, :])
                                if j == 7:
                                    nc.vector.tensor_add(
                                        P2[:, 0, :],
                                        P2[:, 0, :], P2[:, 4, :])
                        use_pe_denom = True
                        for qt in range(2):
                            po = ops.tile([128, 512], f32, tag="o")
                            pd = dps.tile([1, 512], f32, tag="dn")
                            if not use_pe_denom:
                                s1 = at.tile([128, 512], bf16, tag="s1")
                                with nc.allow_low_precision(
                                        reason="bf16 denom partials"):
                                    nc.vector.reduce_sum(
                                        out=s1,
                                        in_=E[:, :, ts(qt, 512)].rearrange(
                                            "p c q -> p q c"),
                                        axis=mybir.AxisListType.X,
                                    )
                            for kc in range(KC):
                                nc.tensor.matmul(
                                    po, vS[:, kc, ts(h, 128)],
                                    E[:, kc, ts(qt, 512)],
                                    start=(kc == 0), stop=(kc == KC - 1),
                                )
                            nc.tensor.matmul(pd, ones_col,
                                             P2[:, 0, ts(qt, 512)],
                                             start=True, stop=True)
                            rd = atr.tile([1, 512], bf16, tag="rd")
                            with nc.allow_low_precision(reason="bf16 denom"):
                                nc.vector.reciprocal(rd, pd)
                            pb = dps.tile([128, 512], f32, tag="dn")
                            nc.tensor.matmul(pb, ones_row, rd,
                                             start=True, stop=True)
                            rb = at.tile([128, 512], f32, tag="rb")
                            nc.vector.tensor_copy(rb, pb)
                            ost = at.tile([128, 512], bf16, tag="ost")
                            nc.vector.tensor_mul(ost, po, rb)
                            nc.sync.dma_start(
                                out=o_d[:, h * N + q0 + qt * 512 :
                                        h * N + q0 + (qt + 1) * 512],
                                in_=ost,
                            )

        # ============ Phase 4: out = o @ (W_out + I) ===================
        if stop_after == 'attn':
            return
        odr = o_d.rearrange("p (h n) -> p h n", h=H)
        with tc.tile_pool(name="ph4o", bufs=3) as ph4o, \
             tc.tile_pool(name="ph4ps", bufs=4, space="PSUM") as ph4ps:
            for t in range(TC):
                oL = ph4o.tile([128, H, 128], bf16, tag="ol")
                nc.sync.dma_start(out=oL, in_=odr[:, :, ts(t, 128)])
                ot = ph4o.tile([128, DIM], f32, tag="ot")
                for n_ in range(4):
                    ps = ph4ps.tile([128, 512], f32)
                    for oc in range(H):
                        nc.tensor.matmul(
                            ps, oL[:, oc, :], Wp[:, oc, ts(n_, 512)],
                            start=(oc == 0), stop=(oc == H - 1),
                        )
                    nc.scalar.copy(ot[:, ts(n_, 512)], ps)
                nc.sync.dma_start(out=out_p[ts(t, 128), :], in_=ot)


def _shard(inputs):
    x = np.ascontiguousarray(inputs["x"], dtype=np.float32)
    emb = np.asarray(inputs["emb"], dtype=np.float32)
    W_emb = np.asarray(inputs["W_emb"], dtype=np.float32)
    b_emb = np.asarray(inputs["b_emb"], dtype=np.float32)
    g_norm = np.asarray(inputs["g_norm"], dtype=np.float32)
    W_qkv = np.ascontiguousarray(inputs["W_qkv"], dtype=np.float32)
    g_q = np.asarray(inputs["g_q"], dtype=np.float32)
    g_k = np.asarray(inputs["g_k"], dtype=np.float32)
    W_out = np.ascontiguousarray(inputs["W_out"], dtype=np.float32)

    # tiny AdaLN conditioning projection done host-side (0.008% of FLOPs):
    # scale_shift = emb @ W_emb + b_emb per batch.
    ss = emb[:, 0, :] @ W_emb + b_emb          # [B, 2*DIM]
    scale, shift = ss[:, :DIM], ss[:, DIM:]
    mcol_b = (g_norm[None, :] * (1.0 + scale)).reshape(B, KC, 128)
    scol_b = shift.reshape(B, KC, 128)
    gqk = np.ascontiguousarray(g_q * g_k)

    in_maps = []
    for core in range(NCORES):
        b, g = core // HG, core % HG
        Wq = W_qkv[:, g * QK : (g + 1) * QK]
        Wk = W_qkv[:, DIM + g * QK : DIM + (g + 1) * QK]
        Wv = W_qkv[:, 2 * DIM + g * QK : 2 * DIM + (g + 1) * QK]
        W_qkv_s = np.ascontiguousarray(np.concatenate([Wq, Wk, Wv], axis=1))
        W_out_s = np.ascontiguousarray(W_out[g * QK : (g + 1) * QK, :]).copy()
        # fold residual: out = o_full @ (W_out + I); this core owns rows
        # g*QK..(g+1)*QK of the identity.
        idx = np.arange(QK)
        W_out_s[idx, g * QK + idx] += 1.0
        in_maps.append({
            "x_b": np.ascontiguousarray(x[b]),
            "mcol_in": np.ascontiguousarray(mcol_b[b].T),
            "scol_in": np.ascontiguousarray(scol_b[b].T),
            "gqk_in": gqk,
            "W_qkv_s": W_qkv_s,
            "W_out_s": W_out_s,
        })
    return in_maps


def get_compiled():
    global _COMPILED
    if _COMPILED is None:
        _COMPILED = _build()
    return _COMPILED


def run_on_hw(inputs, trace=False):
    from concourse.bass_utils import run_bass_kernel_spmd

    nc = get_compiled()
    in_maps = _shard(inputs)
    res = run_bass_kernel_spmd(
        nc, in_maps, core_ids=list(range(NCORES)), trace=trace
    )
    out = np.empty((B, N, DIM), dtype=np.float32)
    for b in range(B):
        out[b] = res.results[HG * b]["out_p"] + res.results[HG * b + 1]["out_p"]
    return out, res


def kernel(**inputs) -> np.ndarray:
    out, _ = run_on_hw(inputs, trace=False)
    return out



# revision 11
# speedup vs baseline: 1.1214x; 1.1214x over previous
# Self-contained Trainium2 Bass kernel for nn_AttentionBlock (AdaLN + QK-norm
# attention), fp8-DoubleRow edition.
#
# Sharding: 8 cores = 4 batches (data parallel) x 2 head-groups of 8 heads
# (tensor parallel).  Each core computes, for its batch b and head group g:
#   xn^T  = (rmsnorm(x_b) * (1+scale) + shift)^T    fp8 e4m3 [dim, n]
#   q,k   = fp8 DR proj -> staged e4m3 -> QK-rmsnorm -> e4m3, d split [64,2,..]
#   v     = fp8 DR proj with e4m3-hi + e5m2-lo weight split  -> e4m3
#   E     = exp(logits/16 - ln32) e4m3; denom via fp8 DR ones-matmul
#   o^T   = fp8 DR (v^T E) * recip(denom), bf16, kept in SBUF
#   out   = o @ (W_out + I) in bf16  -> bf16 partial, host sums the 2 groups
import numpy as np

B, N, DIM = 4, 2048, 2048
H_TOT, D = 16, 128
HG = 2                # head groups (tensor-parallel)
H = H_TOT // HG       # heads per core = 8
QK = H * D            # 1024 q (or k) columns per core
KC = DIM // 128       # 16 dim chunks
KCP = KC // 2         # 8 dim-chunk pairs (DoubleRow)
TC = N // 128         # 16 token chunks
EPS = 1e-6
NCORES = 8
SW = 32.0             # host fp8 scale on W_qkv
SST = 0.25            # q/k psum staging scale
SQK = 4.0             # q/k post-norm fp8 scale
SV = 4.0              # v fp8 scale
LN32 = float(np.log(32.0))

_COMPILED = None


def _build(stop_after=None):
    import concourse.bass as bass
    import concourse.bacc as bacc
    import concourse.tile as tile
    from concourse import mybir
    from concourse.masks import make_identity

    f32 = mybir.dt.float32
    bf16 = mybir.dt.bfloat16
    f8 = mybir.dt.float8e4
    f8l = mybir.dt.float8e5
    AF = mybir.ActivationFunctionType
    OP = mybir.AluOpType

    nc = bacc.Bacc(
        "TRN2", target_bir_lowering=False, debug=False, num_devices=NCORES
    )

    # ---- DRAM I/O -------------------------------------------------------
    x_b = nc.dram_tensor("x_b", [N, DIM], bf16, kind="ExternalInput").ap()
    mcol_in = nc.dram_tensor("mcol_in", [128, KC], f32, kind="ExternalInput").ap()
    scol_in = nc.dram_tensor("scol_in", [128, KC], f32, kind="ExternalInput").ap()
    gqk_in = nc.dram_tensor("gqk_in", [1, D], f32, kind="ExternalInput").ap()
    Wqk8 = nc.dram_tensor("Wqk8", [DIM, 2 * QK], f8, kind="ExternalInput").ap()
    Wvhi8 = nc.dram_tensor("Wvhi8", [DIM, QK], f8, kind="ExternalInput").ap()
    Wvlo8 = nc.dram_tensor("Wvlo8", [DIM, QK], f8l, kind="ExternalInput").ap()
    Wout_s = nc.dram_tensor("Wout_s", [QK, DIM], bf16, kind="ExternalInput").ap()
    out_p = nc.dram_tensor("out_p", [N, DIM], bf16, kind="ExternalOutput").ap()
    dbg = {}
    if stop_after == 'debug':
        dbg['q'] = nc.dram_tensor("qdbg", [128, H * N], f8, kind="ExternalOutput").ap()
        dbg['k'] = nc.dram_tensor("kdbg", [128, H * N], f8, kind="ExternalOutput").ap()
        dbg['v'] = nc.dram_tensor("vdbg", [128, TC * QK], f8, kind="ExternalOutput").ap()
        dbg['xn'] = nc.dram_tensor("xndbg", [128, KC * N], f8, kind="ExternalOutput").ap()
        dbg['e'] = nc.dram_tensor("edbg", [128, KC * 1024], f8, kind="ExternalOutput").ap()
        dbg['o'] = nc.dram_tensor("odbg", [128, H * N], bf16, kind="ExternalOutput").ap()

    with tile.TileContext(nc) as tc:
        _emit(nc, tc, bass, mybir, tile, make_identity, f32, bf16, f8, f8l,
              AF, OP, x_b, mcol_in, scol_in, gqk_in, Wqk8, Wvhi8, Wvlo8,
              Wout_s, out_p, stop_after, dbg)
    nc.compile()
    return nc


def _emit(nc, tc, bass, mybir, tile, make_identity, f32, bf16, f8, f8l,
          AF, OP, x_b, mcol_in, scol_in, gqk_in, Wqk8, Wvhi8, Wvlo8,
          Wout_s, out_p, stop_after=None, dbg=None):
    from contextlib import ExitStack

    ts = bass.ts
    DR = mybir.MatmulPerfMode.DoubleRow
    SNORM = 1.0 / (D * (SW * SST) ** 2)   # Sqrt scale for QK-norm stats
    KROW = (D ** -0.25) * SQK / (SW * SST)

    wqk = Wqk8.rearrange("(c p) n -> p c n", p=128)     # [128, KC, 2*QK]
    wvh = Wvhi8.rearrange("(c p) n -> p c n", p=128)    # [128, KC, QK]
    wvl = Wvlo8.rearrange("(c p) n -> p c n", p=128)
    wout = Wout_s.rearrange("(c p) n -> p c n", p=128)  # [128, H, DIM]

    with ExitStack() as ctx:
        consts = ctx.enter_context(tc.tile_pool(name="consts", bufs=1))

        ident = consts.tile([128, 128], bf16)
        make_identity(nc, ident)
        ones_col = consts.tile([128, 1], bf16)
        nc.vector.memset(ones_col, 1.0)
        ones8b = consts.tile([128, 2, 128], f8)   # DR denominator lhsT
        nc.vector.memset(ones8b, 4.0)             # folds E/32 and vS=4v scales
        krow = consts.tile([1, 128], bf16)
        nc.vector.memset(krow, KROW)
        row025 = consts.tile([1, 128], bf16)
        nc.vector.memset(row025, 0.25)

        mcol = consts.tile([128, KC], f32)          # g_norm*(1+scale) columns
        nc.sync.dma_start(out=mcol, in_=mcol_in)
        scol = consts.tile([128, KC], f32)          # shift columns
        nc.sync.dma_start(out=scol, in_=scol_in)
        gqk_f = consts.tile([1, D], f32)
        nc.sync.dma_start(out=gqk_f, in_=gqk_in)
        gqkrow = consts.tile([1, D], bf16)          # g_q*g_k*D^-.25*SQK/(SW*SST)
        nc.vector.tensor_copy(gqkrow, gqk_f)
        eps128 = consts.tile([128, 1], f32)
        nc.vector.memset(eps128, EPS)
        eps1 = consts.tile([1, 1], f32)
        nc.vector.memset(eps1, EPS)
        ln32n = consts.tile([128, 1], f32)
        nc.vector.memset(ln32n, -LN32)

        # persistent across phases
        vS = consts.tile([128, TC, QK], f8)         # v * SV, token-major
        # d-split q/k for DoubleRow logits, filled by the regroup DMAs below
        spool = ctx.enter_context(tc.tile_pool(name="spool", bufs=1))
        qT8s = spool.tile([64, 2, H, N], f8)
        kT8s = spool.tile([64, 2, H, N], f8)
        if stop_after == 'ph0':
            return

        with ExitStack() as ph2stack:
            xnp = ph2stack.enter_context(tc.tile_pool(name="xnp", bufs=1))
            xnT8 = xnp.tile([128, KC, N], f8)
            stg = ph2stack.enter_context(tc.tile_pool(name="stg", bufs=1))
            qT8f = stg.tile([128, H, N], f8)
            kT8f = stg.tile([128, H, N], f8)

            inner = ph2stack.enter_context(ExitStack())
            ph1 = inner.enter_context(tc.tile_pool(name="ph1", bufs=3))
            ph1s = inner.enter_context(tc.tile_pool(name="ph1s", bufs=4))
            ph1sq = inner.enter_context(tc.tile_pool(name="ph1sq", bufs=1))
            ph2w = inner.enter_context(tc.tile_pool(name="ph2w", bufs=3))
            ph2v = inner.enter_context(tc.tile_pool(name="ph2v", bufs=1))
            p25 = inner.enter_context(tc.tile_pool(name="p25", bufs=2))
            p25r = inner.enter_context(tc.tile_pool(name="p25r", bufs=1))
            ph1ps = inner.enter_context(
                tc.tile_pool(name="ph1ps", bufs=2, space="PSUM"))
            ph2ps = inner.enter_context(
                tc.tile_pool(name="ph2ps", bufs=2, space="PSUM"))
            p25ps = inner.enter_context(
                tc.tile_pool(name="p25ps", bufs=2, space="PSUM"))
            p25bc = inner.enter_context(
                tc.tile_pool(name="p25bc", bufs=2, space="PSUM"))

            def emit_ph1_group(tg):
                # two 2-tile subgroups: halves the first-eviction gate
                for half in range(2):
                    xts, dgs = [], []
                    for tt in range(2):
                        t = tg * 4 + half * 2 + tt
                        xt = ph1.tile([128, DIM], bf16, tag="xt", name=f"xt{t}")
                        nc.gpsimd.dma_start(out=xt, in_=x_b[ts(t, 128), :])
                        sq = ph1sq.tile([128, DIM], bf16, tag="sq",
                                        name=f"sq{t}")
                        ssq = ph1s.tile([128, 1], f32, tag="ssq", name=f"ssq{t}")
                        nc.scalar.activation(sq, xt, AF.Square, accum_out=ssq)
                        rin = ph1s.tile([128, 1], f32, tag="rin", name=f"ri{t}")
                        nc.scalar.activation(rin, ssq, AF.Sqrt,
                                             scale=1.0 / DIM, bias=eps128)
                        rr = ph1s.tile([128, 1], f32, tag="rr", name=f"rr{t}")
                        nc.vector.reciprocal(rr, rin)
                        diag = ph1s.tile([128, 128], bf16, tag="dg",
                                         name=f"dg{t}")
                        nc.vector.tensor_scalar_mul(diag, ident, rr)
                        xts.append(xt)
                        dgs.append(diag)
                    for c in range(KC):
                        pst = ph1ps.tile([128, 256], f32, tag="pt",
                                         name=f"pt{tg}_{half}_{c}")
                        for tt in range(2):
                            nc.tensor.matmul(pst[:, ts(tt, 128)],
                                             xts[tt][:, ts(c, 128)],
                                             dgs[tt],
                                             start=True, stop=True)
                        with nc.allow_low_precision(reason="fp8 xn"):
                            nc.vector.tensor_scalar(
                                out=xnT8[:, c,
                                         tg * 512 + half * 256 :
                                         tg * 512 + (half + 1) * 256],
                                in0=pst,
                                scalar1=mcol[:, c : c + 1],
                                scalar2=scol[:, c : c + 1],
                                op0=OP.mult, op1=OP.add,
                            )

            def emit_qk_mchunk(m, wm=None):
                # m in 0..15: even slots = q head m//2... we use explicit map:
                # column chunk m of Wqk8; m < H -> q head m, else k head m-H.
                if wm is None:
                    wm = ph2w.tile([128, KC, 128], f8, tag="wqk",
                                   name=f"wm{m}")
                    nc.gpsimd.dma_start(out=wm, in_=wqk[:, :, ts(m, 128)])
                dst = qT8f if m < H else kT8f
                hh = m if m < H else m - H
                for nt in range(4):
                    ps = ph2ps.tile([128, 512], f32, tag="qkps",
                                    name=f"qk{m}_{nt}")
                    for cp in range(KCP):
                        nc.tensor.matmul(
                            ps, wm[:, 2 * cp : 2 * cp + 2, :],
                            xnT8[:, 2 * cp : 2 * cp + 2, ts(nt, 512)],
                            start=(cp == 0), stop=(cp == KCP - 1),
                            perf_mode=DR,
                        )
                    with nc.allow_low_precision(reason="fp8 qk stage"):
                        nc.vector.tensor_scalar_mul(
                            dst[:, hh, ts(nt, 512)], ps, SST)

            def emit_ph25_head(h):
                for src8, lrow in ((qT8f, gqkrow), (kT8f, krow)):
                    sq = p25.tile([128, N], bf16, tag="sq25",
                                  name=f"sq25_{h}")
                    nc.vector.tensor_mul(sq, src8[:, h, :], src8[:, h, :])
                    rq1 = p25r.tile([1, N], bf16, tag="rq", name=f"rq{h}")
                    for nt in range(4):
                        pss = p25ps.tile([1, 512], f32, tag="sm")
                        nc.tensor.matmul(pss, ones_col, sq[:, ts(nt, 512)],
                                         start=True, stop=True)
                        nc.scalar.activation(rq1[:, ts(nt, 512)], pss,
                                             AF.Sqrt, scale=SNORM, bias=eps1)
                    rv = p25r.tile([1, N], bf16, tag="rv", name=f"rv{h}")
                    with nc.allow_low_precision(reason="bf16 rms scale"):
                        nc.vector.reciprocal(rv, rq1)
                    for nt in range(4):
                        pb = p25bc.tile([128, 512], f32, tag="bc")
                        nc.tensor.matmul(pb, lrow, rv[:, ts(nt, 512)],
                                         start=True, stop=True)
                        with nc.allow_low_precision(reason="fp8 qk norm"):
                            nc.vector.tensor_mul(
                                src8[:, h, ts(nt, 512)],
                                src8[:, h, ts(nt, 512)], pb)

            def emit_v_slab(nv):
                wvh_t = ph2v.tile([128, KC, 512], f8, tag="wvh",
                                  name=f"wvh{nv}")
                nc.gpsimd.dma_start(out=wvh_t, in_=wvh[:, :, ts(nv, 512)])
                wvl_t = ph2v.tile([128, KC, 512], f8l, tag="wvl",
                                  name=f"wvl{nv}")
                nc.gpsimd.dma_start(out=wvl_t, in_=wvl[:, :, ts(nv, 512)])
                for t in range(TC):
                    ps = ph2ps.tile([128, 512], f32, tag="qkps",
                                    name=f"v{nv}_{t}")
                    for cp in range(KCP):
                        nc.tensor.matmul(
                            ps, xnT8[:, 2 * cp : 2 * cp + 2, ts(t, 128)],
                            wvh_t[:, 2 * cp : 2 * cp + 2, :],
                            start=(cp == 0), stop=False, perf_mode=DR,
                        )
                    for cp in range(KCP):
                        nc.tensor.matmul(
                            ps, xnT8[:, 2 * cp : 2 * cp + 2, ts(t, 128)],
                            wvl_t[:, 2 * cp : 2 * cp + 2, :],
                            start=False, stop=(cp == KCP - 1), perf_mode=DR,
                        )
                    with nc.allow_low_precision(reason="fp8 v"):
                        nc.vector.tensor_scalar_mul(
                            vS[:, t, ts(nv, 512)], ps, SV / SW)

            # ---- emit order: ph1 feeds proj; heads pipelined -------------
            for tg in range(4):
                emit_ph1_group(tg)
            m_order = [x for h in range(H) for x in (h, H + h)]
            for i, m in enumerate(m_order):
                emit_qk_mchunk(m)
                if i % 2 == 1:
                    emit_ph25_head(m - H)
                if i == 7:
                    emit_v_slab(0)
                if i == 11:
                    emit_v_slab(1)

            if stop_after == 'ph2a':
                return

            if dbg:
                nc.sync.dma_start(out=dbg['q'], in_=qT8f.rearrange("p h n -> p (h n)"))
                nc.sync.dma_start(out=dbg['k'], in_=kT8f.rearrange("p h n -> p (h n)"))
                nc.sync.dma_start(out=dbg['v'], in_=vS.rearrange("p t c -> p (t c)"))
                nc.sync.dma_start(out=dbg['xn'], in_=xnT8.rearrange("p c n -> p (c n)"))
            inner.close()
            # d-split regroup for DoubleRow logits: [128,H,N] -> [64,2,H,N]
            for src, dstt in ((qT8f, qT8s), (kT8f, kT8s)):
                nc.sync.dma_start(out=dstt[:, 0], in_=src[0:64])
                nc.sync.dma_start(out=dstt[:, 1], in_=src[64:128])

        if stop_after == 'regroup':
            return

        # ---- W_out prefetch; o^T stays in SBUF --------------------------
        ph4w = ctx.enter_context(tc.tile_pool(name="ph4w", bufs=1,
                                              side="right"))
        wops = []
        oTp = ctx.enter_context(tc.tile_pool(name="oTp", bufs=1))
        oT = oTp.tile([128, H, N], bf16)

        # ======== Phase 3: attention ===============================
        with tc.tile_pool(name="Ep", bufs=1) as Ep, \
             tc.tile_pool(name="at", bufs=2) as at, \
             tc.tile_pool(name="lps", bufs=2, space="PSUM") as lps, \
             tc.tile_pool(name="ops", bufs=2, space="PSUM") as ops, \
             tc.tile_pool(name="dps", bufs=1, space="PSUM") as dps:
            E = Ep.tile([128, KC, 1024], f8)
            for n_ in range(4):
                wop = ph4w.tile([128, H, 512], bf16, name=f"wop{n_}")
                nc.sync.dma_start(out=wop, in_=wout[:, :, ts(n_, 512)])
                wops.append(wop)
            for h in range(H):
                for qh in range(2):
                    q0 = qh * 1024
                    pd = dps.tile([128, 1024], f32, tag="dn",
                                  name=f"pd{h}_{qh}")
                    for kc in range(KC):
                        pl = lps.tile([128, 1024], f32, tag="lg")
                        for j in range(2):
                            nc.tensor.matmul(
                                pl[:, ts(j, 512)],
                                kT8s[:, :, h, ts(kc, 128)],
                                qT8s[:, :, h,
                                     q0 + j * 512 : q0 + (j + 1) * 512],
                                start=True, stop=True, perf_mode=DR,
                            )
                        nc.scalar.activation(E[:, kc, :], pl, AF.Exp,
                                             scale=1.0 / (SQK * SQK),
                                             bias=ln32n)
                        if kc % 2 == 1:
                            for qt in range(2):
                                nc.tensor.matmul(
                                    pd[:, ts(qt, 512)], ones8b,
                                    E[:, kc - 1 : kc + 1, ts(qt, 512)],
                                    start=(kc == 1), stop=(kc == KC - 1),
                                    perf_mode=DR,
                                )
                    if dbg and h == 0 and qh == 0:
                        nc.sync.dma_start(out=dbg['e'], in_=E.rearrange("p c n -> p (c n)"))
                    for qt in range(2):
                        po = ops.tile([128, 512], f32, tag="o")
                        for cp in range(KCP):
                            nc.tensor.matmul(
                                po, vS[:, 2 * cp : 2 * cp + 2, ts(h, 128)],
                                E[:, 2 * cp : 2 * cp + 2, ts(qt, 512)],
                                start=(cp == 0), stop=(cp == KCP - 1),
                                perf_mode=DR,
                            )
                        rb = at.tile([128, 512], f32, tag="rb")
                        nc.vector.reciprocal(rb, pd[:, ts(qt, 512)])
                        nc.vector.tensor_mul(
                            oT[:, h, q0 + qt * 512 : q0 + (qt + 1) * 512],
                            po, rb)

        if dbg:
            nc.sync.dma_start(out=dbg['o'], in_=oT.rearrange("p h n -> p (h n)"))
        # ============ Phase 4: out = o @ (W_out + I) ===================
        if stop_after == 'attn':
            return
        with tc.tile_pool(name="ph4o", bufs=3) as ph4o, \
             tc.tile_pool(name="ph4ps", bufs=4, space="PSUM") as ph4ps:
            for n_ in range(4):
                for t in range(TC):
                    ps = ph4ps.tile([128, 512], f32)
                    for oc in range(H):
                        nc.tensor.matmul(
                            ps, oT[:, oc, ts(t, 128)], wops[n_][:, oc, :],
                            start=(oc == 0), stop=(oc == H - 1),
                        )
                    ot = ph4o.tile([128, 512], bf16, tag="ot")
                    nc.vector.tensor_copy(ot, ps)
                    nc.sync.dma_start(
                        out=out_p[ts(t, 128), ts(n_, 512)], in_=ot)


def _shard(inputs):
    import ml_dtypes
    BF = ml_dtypes.bfloat16
    F8 = ml_dtypes.float8_e4m3
    F8L = ml_dtypes.float8_e5m2

    x = np.ascontiguousarray(inputs["x"], dtype=np.float32)
    emb = np.asarray(inputs["emb"], dtype=np.float32)
    W_emb = np.asarray(inputs["W_emb"], dtype=np.float32)
    b_emb = np.asarray(inputs["b_emb"], dtype=np.float32)
    g_norm = np.asarray(inputs["g_norm"], dtype=np.float32)
    W_qkv = np.ascontiguousarray(inputs["W_qkv"], dtype=np.float32)
    g_q = np.asarray(inputs["g_q"], dtype=np.float32)
    g_k = np.asarray(inputs["g_k"], dtype=np.float32)
    W_out = np.ascontiguousarray(inputs["W_out"], dtype=np.float32)

    # tiny AdaLN conditioning projection done host-side (0.008% of FLOPs)
    ss = emb[:, 0, :] @ W_emb + b_emb          # [B, 2*DIM]
    scale, shift = ss[:, :DIM], ss[:, DIM:]
    mcol_b = (g_norm[None, :] * (1.0 + scale)).reshape(B, KC, 128)
    scol_b = shift.reshape(B, KC, 128)
    gqk = ((g_q * g_k) * (D ** -0.25) * SQK / (SW * SST)).astype(np.float32)

    in_maps = []
    for core in range(NCORES):
        b, g = core // HG, core % HG
        Wq = W_qkv[:, g * QK : (g + 1) * QK]
        Wk = W_qkv[:, DIM + g * QK : DIM + (g + 1) * QK]
        Wv = W_qkv[:, 2 * DIM + g * QK : 2 * DIM + (g + 1) * QK]
        Wqk8 = np.ascontiguousarray(
            np.concatenate([Wq, Wk], axis=1) * SW).astype(F8)
        Wvf = np.ascontiguousarray(Wv * SW)
        Wvhi8 = Wvf.astype(F8)
        Wvlo8 = (Wvf - Wvhi8.astype(np.float32)).astype(F8L)
        W_out_s = np.ascontiguousarray(W_out[g * QK : (g + 1) * QK, :]).copy()
        # fold residual: out = o_full @ (W_out + I); this core owns rows
        # g*QK..(g+1)*QK of the identity.
        idx = np.arange(QK)
        W_out_s[idx, g * QK + idx] += 1.0
        in_maps.append({
            "x_b": np.ascontiguousarray(x[b]).astype(BF),
            "mcol_in": np.ascontiguousarray(mcol_b[b].T),
            "scol_in": np.ascontiguousarray(scol_b[b].T),
            "gqk_in": np.ascontiguousarray(gqk[None, :]),
            "Wqk8": Wqk8,
            "Wvhi8": Wvhi8,
            "Wvlo8": Wvlo8,
            "Wout_s": W_out_s.astype(BF),
        })
    return in_maps


def get_compiled():
    global _COMPILED
    if _COMPILED is None:
        _COMPILED = _build()
    return _COMPILED


def run_on_hw(inputs, trace=False):
    from concourse.bass_utils import run_bass_kernel_spmd

    nc = get_compiled()
    in_maps = _shard(inputs)
    res = run_bass_kernel_spmd(
        nc, in_maps, core_ids=list(range(NCORES)), trace=trace
    )
    out = np.empty((B, N, DIM), dtype=np.float32)
    for b in range(B):
        out[b] = (res.results[HG * b]["out_p"].astype(np.float32)
                  + res.results[HG * b + 1]["out_p"].astype(np.float32))
    return out, res


def kernel(**inputs) -> np.ndarray:
    out, _ = run_on_hw(inputs, trace=False)
    return out
